# revision 1
# baseline (speedup 1.0000x reference)
"""Trainium2 Bass kernel for nn_Conv2dTB (BN -> ternary quantize -> 3x3 conv
-> beta box-filter scaling), data-parallel over batch on 8 NeuronCores.

Contract: kernel(**inputs) takes the FULL unsharded inputs as numpy arrays and
returns the FULL [16, 256, 56, 56] float32 output. Internally the batch dim is
split 2 images/core; BN batch statistics use an on-device AllReduce so
normalization matches the reference's full-batch statistics.

v2 structure vs v1:
 - x loads split across the two hardware DGE queues (sync + scalar engines),
   4 large descriptors per queue, in uneven slabs (3/4 + 1/4) so the last BN
   stat chunk is small and the stats tail after the final DMA is short.
 - BN stats: ACT owns sum(x^2) via Square+accum (single table), DVE owns
   sum(x); chunks pipeline with the DMA stream.
 - Weight f32->f16 conversion done by the casting Pool-queue DMA (no f32
   staging buffer); PSUM->SBUF weight-transpose copies on DVE fill the
   collective wait window.
 - beta map broadcast to 128 partitions via a K=1 matmul from an x-major
   flattened single-partition row (flatten is one casting Pool DMA straight
   from the transposed map) instead of a per-tile DRAM-bounce broadcast DMA.
 - Per-image interleave: beta chain (channel-sum matmuls, box filter,
   flatten) for an image is emitted with that image's conv loop, so image 1
   prerequisites never stall the PE queue during image 0's conv.
 - scale/shift computed on both channel blocks at once (7 DVE ops + 1 ACT
   sqrt); Sign ternarization in quarter-row chunks so the first conv matmul
   starts ~2us after the collective lands.
"""

import numpy as np

# Problem shapes (hardcoded per contract).
N, C, H, W = 16, 256, 56, 56
COUT = 256
KS = 3
EPS = 1e-4
N_CORES = 8
NLOC = N // N_CORES  # images per core (2)
CB = C // 128  # channel blocks (2)
COB = COUT // 128  # cout blocks (2)
RT_ROWS = 8  # image rows per pixel tile
NT = H // RT_ROWS  # row tiles per image (7)
NPIX = RT_ROWS * W  # pixels per tile (448)
HW = H * W  # 3136
Q4 = HW // 4  # stats chunk (784 pixels -> 784 f32 per partition)
PH = H + 2  # padded rows (58)
PW = W + 2  # padded cols (58)
COUNT = float(N * H * W)  # BN reduction count (full batch)
BF = 3200  # padded flat beta row stride

COLLECTIVE = "allreduce"  # or "allgather"

_CACHE = {}


def _build():
    import concourse.tile as tile
    from concourse import bacc, mybir

    f32 = mybir.dt.float32
    f16 = mybir.dt.float16
    AF = mybir.ActivationFunctionType
    ALU = mybir.AluOpType

    nc = bacc.Bacc("TRN2", target_bir_lowering=False, debug=False,
                   num_devices=N_CORES)

    # ---- external I/O ----
    x_d = nc.dram_tensor("x", [NLOC, C, H, W], f32, kind="ExternalInput").ap()
    gamma_d = nc.dram_tensor("bn_gamma", [C], f32, kind="ExternalInput").ap()
    bnbeta_d = nc.dram_tensor("bn_beta", [C], f32, kind="ExternalInput").ap()
    w_d = nc.dram_tensor("conv_w", [COUT, C, KS, KS], f32,
                         kind="ExternalInput").ap()
    cb_d = nc.dram_tensor("conv_b", [COUT], f32, kind="ExternalInput").ap()
    bb_d = nc.dram_tensor("beta_conv_b", [1], f32, kind="ExternalInput").ap()
    # host-provided constants (ident56 unused in v2 but kept for the harness)
    ident_d = nc.dram_tensor("ident128", [128, 128], f32,
                             kind="ExternalInput").ap()
    nc.dram_tensor("ident56", [H, H], f32, kind="ExternalInput")
    t3_d = nc.dram_tensor("tridiag", [H, H], f32, kind="ExternalInput").ap()
    cnt_d = nc.dram_tensor("boxcnt", [H, W], f32, kind="ExternalInput").ap()
    out_d = nc.dram_tensor("out", [NLOC, COUT, H, W], f32,
                           kind="ExternalOutput").ap()

    import concourse.bass as bass

    with tile.TileContext(nc) as tc:
        with (
            tc.tile_pool(name="persist", bufs=1) as persist,
            tc.tile_pool(name="scratch", bufs=2) as scratch,
            tc.tile_pool(name="stage", bufs=3) as stage,
            tc.tile_pool(name="outp", bufs=4) as outp,
            tc.tile_pool(name="ps_y", bufs=6, space="PSUM") as ps_y,
            tc.tile_pool(name="ps_b", bufs=1, space="PSUM") as ps_b,
            tc.tile_pool(name="ps_m", bufs=1, space="PSUM") as ps_m,
            tc.tile_pool(name="dram", bufs=1, space="DRAM") as dram,
        ):
            # ---------------- small const loads (gpsimd queue) ------------
            ident_sb = persist.tile([128, 128], f32)
            nc.gpsimd.dma_start(out=ident_sb[:], in_=ident_d[:])
            t3_sb = persist.tile([H, H], f32)
            nc.gpsimd.dma_start(out=t3_sb[:], in_=t3_d[:])
            cnt_sb = persist.tile([H, W], f32)
            nc.gpsimd.dma_start(out=cnt_sb[:], in_=cnt_d[:])
            gamma_sb = persist.tile([128, CB], f32)
            nc.gpsimd.dma_start(out=gamma_sb[:],
                                in_=gamma_d.rearrange("(cb p) -> p cb", p=128))
            bnbeta_sb = persist.tile([128, CB], f32)
            nc.gpsimd.dma_start(out=bnbeta_sb[:],
                                in_=bnbeta_d.rearrange("(cb p) -> p cb", p=128))
            convb_cols = persist.tile([128, COB], f32)
            nc.gpsimd.dma_start(out=convb_cols[:],
                                in_=cb_d.rearrange("(cob p) -> p cob", p=128))
            bb56 = persist.tile([H, 1], f32)
            bbsrc = bb_d[0:1]
            nc.gpsimd.dma_start(
                out=bb56[:],
                in_=bass.AP(tensor=bbsrc.tensor, offset=bbsrc.offset,
                            ap=[[0, H], [1, 1]]))

            # ---------------- x loads: 2 HW queues, half slabs ------------
            # sync queue carries img0, scalar queue carries img1. 4 big
            # descriptors per queue (per-descriptor ring overhead ~2.5us).
            x_sb = persist.tile([128, NLOC, CB, HW], f32)
            xv = [x_d[img].rearrange("(cb p) h w -> cb p (h w)", p=128)
                  for img in range(NLOC)]
            CUT = 3 * HW // 4
            for h in range(2):
                sl = slice(0, CUT) if h == 0 else slice(CUT, HW)
                for cbk in range(CB):
                    nc.sync.dma_start(out=x_sb[:, 0, cbk, sl],
                                      in_=xv[0][cbk][:, sl])
                    if cbk == 0:
                        nc.scalar.dma_start(out=x_sb[:, 1, cbk, sl],
                                            in_=xv[1][cbk][:, sl])
                    else:
                        nc.gpsimd.dma_start(out=x_sb[:, 1, cbk, sl],
                                            in_=xv[1][cbk][:, sl])

            # ---------------- BN partial stats ----------------------------
            # ACT owns sum(x^2) (Square fused accum, one table), DVE owns
            # sum(x). layout: [128, kind(2: sx, sq), cb, img*2+h]
            stats = persist.tile([128, 2, CB, NLOC * 2], f32)
            for h in range(2):
                sl = slice(0, CUT) if h == 0 else slice(CUT, HW)
                ln = sl.stop - sl.start
                for img in range(NLOC):
                    for cbk in range(CB):
                        xs = x_sb[:, img, cbk, sl]
                        col = img * 2 + h
                        nc.vector.reduce_sum(stats[:, 0, cbk, col:col + 1],
                                             xs, axis=mybir.AxisListType.X)
                        sq_junk = scratch.tile([128, ln], f32,
                                               tag=f"sqj{h}", name="sqj")
                        nc.scalar.activation(
                            sq_junk[:], xs, AF.Square,
                            accum_out=stats[:, 1, cbk, col:col + 1])

            partial = persist.tile([128, 2, CB], f32)
            for k in range(2):
                for cbk in range(CB):
                    nc.vector.reduce_sum(partial[:, k, cbk:cbk + 1],
                                         stats[:, k, cbk, :],
                                         axis=mybir.AxisListType.X)

            # ---------------- weight prep (PE + Pool; ACT stays free) -----
            w_bf = persist.tile([128, COB, C, KS * KS], f16)
            wv = w_d.rearrange("(cob p) c k1 k2 -> cob p c (k1 k2)", p=128)
            for cob in range(COB):
                for a in range(2):
                    sl = slice(a * (C // 2), (a + 1) * (C // 2))
                    nc.gpsimd.dma_start(out=w_bf[:, cob, sl, :],
                                        in_=wv[cob][:, sl, :])
            ident_bf = persist.tile([128, 128], f16)
            nc.gpsimd.tensor_copy(ident_bf[:], ident_sb[:])
            wT = persist.tile([128, CB, KS * KS, COB, 128], f16)
            for cob in range(COB):
                for cbk in range(CB):
                    for tap in range(KS * KS):
                        wsl = w_bf[:, cob, cbk * 128:(cbk + 1) * 128, tap]
                        ps_t = ps_m.tile([128, 128], f16, tag="psm")
                        nc.tensor.transpose(ps_t[:], wsl, ident_bf[:])
                        nc.vector.tensor_copy(wT[:, cbk, tap, cob, :], ps_t[:])

            t_pad = persist.tile([128, CB, NLOC, PH, PW], f16)

            # ---------------- collective: stats across the 8 cores --------
            allred = persist.tile([128, 2, CB], f32)
            if COLLECTIVE == "allgather":
                bounce_in = dram.tile([1, 512], f32)
                bounce_out = dram.tile([8, 512], f32)
                nc.sync.dma_start(out=bounce_in.rearrange("o (p f) -> p o f",
                                                          p=128)[:],
                                  in_=partial[:])
                nc.gpsimd.collective_compute(
                    "AllGather", mybir.AluOpType.bypass,
                    replica_groups=[list(range(N_CORES))],
                    ins=[bounce_in.opt()], outs=[bounce_out.opt()],
                )
                slots = persist.tile([128, 8, 4], f32)
                nc.sync.dma_start(
                    out=slots[:],
                    in_=bounce_out.rearrange("s (p f) -> p s f", p=128)[:])
                sfl = slots.rearrange("p s f -> p (s f)")
                nc.vector.tensor_add(sfl[:, 0:16], sfl[:, 0:16], sfl[:, 16:32])
                nc.vector.tensor_add(sfl[:, 0:8], sfl[:, 0:8], sfl[:, 8:16])
                nc.vector.tensor_add(
                    allred.rearrange("p k c -> p (k c)")[:],
                    sfl[:, 0:4], sfl[:, 4:8])
            else:
                bounce_in = dram.tile([128, 4], f32)
                bounce_out = dram.tile([128, 4], f32)
                nc.sync.dma_start(
                    out=bounce_in[:],
                    in_=partial.rearrange("p k c -> p (k c)")[:])
                nc.gpsimd.collective_compute(
                    "AllReduce", mybir.AluOpType.add,
                    replica_groups=[list(range(N_CORES))],
                    ins=[bounce_in.opt()], outs=[bounce_out.opt()],
                )
                nc.sync.dma_start(
                    out=allred.rearrange("p k c -> p (k c)")[:],
                    in_=bounce_out[:])

            # Emitted after the collective so the Pool FIFO reaches the
            # trigger as soon as the stats semaphore fires; these all run
            # during the AllReduce wait window.
            # t_pad zero borders (Pool)
            for cbk in range(CB):
                for img in range(NLOC):
                    nc.gpsimd.memset(t_pad[:, cbk, img, 0, :], 0.0)
                    nc.gpsimd.memset(t_pad[:, cbk, img, PH - 1, :], 0.0)
                    nc.gpsimd.memset(t_pad[:, cbk, img, 1:PH - 1, 0], 0.0)
                    nc.gpsimd.memset(t_pad[:, cbk, img, 1:PH - 1, PW - 1], 0.0)
            # ones for the K=1 beta broadcast matmul
            ones16 = persist.tile([1, 128], f16)
            nc.gpsimd.memset(ones16[:], 1.0)
            ones_c = persist.tile([128, 1], f16)
            nc.gpsimd.memset(ones_c[:], 1.0)
            # 1 / (256 * boxcount + beta_conv_b)
            den56 = persist.tile([H, W], f32)
            nc.gpsimd.tensor_scalar(den56[:], cnt_sb[:], 256.0, bb56[:],
                                    ALU.mult, ALU.add)
            invden = persist.tile([H, W], f32)
            nc.vector.reciprocal(invden[:], den56[:])

            # scale/shift, both cb columns at once: xn = x*scale + shift
            scale = persist.tile([128, CB], f32)
            shift = persist.tile([128, CB], f32)
            mean = stage.tile([128, CB], f32, tag="mean")
            nc.vector.tensor_scalar_mul(mean[:], allred[:, 0, :], 1.0 / COUNT)
            ex2e = stage.tile([128, CB], f32, tag="ex2e")
            nc.vector.tensor_scalar(ex2e[:], allred[:, 1, :], 1.0 / COUNT,
                                    EPS, ALU.mult, ALU.add)
            msq = stage.tile([128, CB], f32, tag="msq")
            nc.vector.tensor_mul(msq[:], mean[:], mean[:])
            var = stage.tile([128, CB], f32, tag="var")
            nc.vector.tensor_sub(var[:], ex2e[:], msq[:])
            rvar = stage.tile([128, CB], f32, tag="rvar")
            nc.vector.reciprocal(rvar[:], var[:])
            rstd = stage.tile([128, CB], f32, tag="rstd")
            nc.scalar.sqrt(rstd[:], rvar[:])
            nc.vector.tensor_mul(scale[:], rstd[:], gamma_sb[:])
            ms = stage.tile([128, CB], f32, tag="ms")
            nc.vector.tensor_mul(ms[:], mean[:], scale[:])
            nc.vector.tensor_sub(shift[:], bnbeta_sb[:], ms[:])

            # ---------------- ternarize (ACT) + clip-abs (DVE) ------------
            # ACT order: sign(img0) first (unblocks the conv), then abs for
            # BOTH images (feeds both beta chains early), then sign(img1)
            # (not needed until img1's conv, ~70us later).
            xq = H // 4

            def emit_signs(img):
                for quar in range(4):
                    rs = slice(quar * xq, (quar + 1) * xq)
                    prs = slice(1 + quar * xq, 1 + (quar + 1) * xq)
                    for cbk in range(CB):
                        tv = t_pad[:, cbk, img, prs, 1:PW - 1]
                        nc.scalar.activation(
                            tv,
                            x_sb[:, img, cbk, :].rearrange(
                                "p (h w) -> p h w", w=W)[:, rs, :],
                            AF.Sign, bias=shift[:, cbk:cbk + 1],
                            scale=scale[:, cbk:cbk + 1])

            def emit_abs(img):
                for cbk in range(CB):
                    ab_t = scratch.tile([128, HW], f32, tag="abt", name="abt")
                    nc.scalar.activation(ab_t[:], x_sb[:, img, cbk, :],
                                         AF.Abs, bias=shift[:, cbk:cbk + 1],
                                         scale=scale[:, cbk:cbk + 1])
                    nc.vector.tensor_scalar_min(xc_sb[:, cbk, img, :],
                                                ab_t[:], 1.0)
                nc.vector.tensor_add(c2_sb[:, img, :], xc_sb[:, 0, img, :],
                                     xc_sb[:, 1, img, :])

            xc_sb = persist.tile([128, CB, NLOC, HW], f16)
            c2_sb = persist.tile([128, NLOC, HW], f16)
            emit_signs(0)
            emit_abs(0)
            emit_abs(1)
            emit_signs(1)

            # ---------------- beta map -> flat single-partition rows ------
            bflat = persist.tile([1, NLOC, BF], f16)
            cT_grid = persist.tile([H, NLOC, PW], f32)
            for img in range(NLOC):
                nc.vector.memset(cT_grid[:, img, 0:1], 0.0)
                nc.vector.memset(cT_grid[:, img, PW - 1:PW], 0.0)

            # ---------------- per image: beta chain then conv -------------
            ov = out_d.rearrange("n (cob p) h w -> n cob p (h w)", p=128)
            for img in range(NLOC):
                for rt in range(NT):
                    pct = ps_m.tile([H, RT_ROWS], f32, tag="psm")
                    for r in range(RT_ROWS):
                        y = rt * RT_ROWS + r
                        nc.tensor.matmul(
                            pct[:, r:r + 1],
                            c2_sb[:, img, y * W:(y + 1) * W],
                            ones_c[:], start=True, stop=True)
                    nc.vector.tensor_copy(
                        cT_grid[:, img,
                                1 + rt * RT_ROWS:1 + (rt + 1) * RT_ROWS],
                        pct[:])

                # box over y (free dim), then over x via tridiagonal matmul
                hsumT = stage.tile([H, W], f32, tag="hsumT")
                cg = cT_grid[:, img, :]
                nc.vector.tensor_add(hsumT[:], cg[:, 0:W], cg[:, 1:W + 1])
                nc.vector.tensor_add(hsumT[:], hsumT[:], cg[:, 2:W + 2])
                pbT = ps_m.tile([H, W], f32, tag="psm")
                nc.tensor.matmul(pbT[:], t3_sb[:], hsumT[:], start=True,
                                 stop=True)
                bmapT = stage.tile([H, W], f32, tag="bmapT")
                nc.vector.tensor_scalar_add(bmapT[:], pbT[:], bb56[:])
                nc.vector.tensor_mul(bmapT[:], bmapT[:], invden[:])
                # flatten x-major: bflat[0, x*56+y] = bmapT[x, y], f32->f16
                # cast on the Pool software-DGE queue
                bsl = bflat[0:1, img, 0:HW].rearrange("p (x y) -> p x y", y=H)
                nc.gpsimd.dma_start(out=bsl[:], in_=bmapT[:])

                for rt in range(NT):
                    pbb = ps_b.tile([128, NPIX], f32)
                    bfv = bflat[0:1, img, 0:HW].rearrange(
                        "p (x y) -> p y x", y=H)
                    nc.tensor.matmul(
                        pbb[:], ones16[:],
                        bfv[:, rt * RT_ROWS:(rt + 1) * RT_ROWS, :],
                        start=True, stop=True)
                    bbc = outp.tile([128, NPIX], f32, tag="bbc")
                    nc.scalar.copy(bbc[:], pbb[:])
                    for cob in range(COB):
                        py = ps_y.tile([128, NPIX], f32)
                        first = True
                        for cbk in range(CB):
                            for ky in range(KS):
                                for kx in range(KS):
                                    rhs = t_pad[:, cbk, img,
                                                rt * RT_ROWS + ky:
                                                rt * RT_ROWS + ky + RT_ROWS,
                                                kx:kx + W]
                                    last = (cbk == CB - 1 and ky == KS - 1
                                            and kx == KS - 1)
                                    nc.tensor.matmul(
                                        py[:],
                                        wT[:, cbk, ky * KS + kx, cob, :],
                                        rhs, start=first, stop=last)
                                    first = False
                        # out = (conv + bias) * beta in one pass; f16 staging
                        # halves DVE time and SBUF read traffic, the casting
                        # Pool-queue DMA expands to f32 on store
                        osb = outp.tile([128, NPIX], f16, tag="osb")
                        nc.vector.scalar_tensor_tensor(
                            osb[:], py[:], convb_cols[:, cob:cob + 1], bbc[:],
                            ALU.add, ALU.mult)
                        nc.gpsimd.dma_start(
                            out=ov[img, cob][:, rt * NPIX:(rt + 1) * NPIX],
                            in_=osb[:])

    nc.compile()
    return nc


def _consts():
    ident = np.eye(128, dtype=np.float32)
    ident56 = np.eye(H, dtype=np.float32)
    t3 = np.zeros((H, H), dtype=np.float32)
    for i in range(H):
        for j in range(max(0, i - 1), min(H, i + 2)):
            t3[j, i] = 1.0
    r = np.minimum(np.arange(H), H - 1 - np.arange(H))
    edge = (r >= 1).astype(np.float32) + 2.0  # 2 on border rows, 3 inside
    cnt = np.outer(edge, edge).astype(np.float32)  # valid taps: 4/6/9
    return ident, ident56, t3, cnt


def kernel(**inputs):
    from concourse.bass_utils import run_bass_kernel_spmd

    if "nc" not in _CACHE:
        _CACHE["nc"] = _build()
    nc = _CACHE["nc"]

    x = np.ascontiguousarray(inputs["x"], dtype=np.float32)
    ident, ident56, t3, cnt = _consts()
    shared = {
        "bn_gamma": np.ascontiguousarray(inputs["bn_gamma"], np.float32),
        "bn_beta": np.ascontiguousarray(inputs["bn_beta"], np.float32),
        "conv_w": np.ascontiguousarray(inputs["conv_w"], np.float32),
        "conv_b": np.ascontiguousarray(inputs["conv_b"], np.float32),
        "beta_conv_b": np.ascontiguousarray(inputs["beta_conv_b"], np.float32),
        "ident128": ident, "ident56": ident56, "tridiag": t3, "boxcnt": cnt,
    }
    in_maps = [
        {"x": np.ascontiguousarray(x[i * NLOC:(i + 1) * NLOC]), **shared}
        for i in range(N_CORES)
    ]
    res = run_bass_kernel_spmd(nc, in_maps, list(range(N_CORES)))
    out = np.concatenate([res.results[i]["out"] for i in range(N_CORES)],
                         axis=0)
    return out.astype(np.float32)

